# revision 19
# baseline (speedup 1.0000x reference)
"""EnhancedMultiHeadAttention on 8 TRN2 NeuronCores.

Sharding: core c handles batch b=c//2 and query-row half h=c%2.
Each core computes the full attention for its 1024 query rows against its
batch's full 2048 keys/values (k/v work duplicated across the 2 cores that
share a batch — cheaper than an all-reduce). Outputs are disjoint slices of
the full [4, 2048, 1024] result, assembled on the host.

Kernel structure per core (bf16 matmuls, f32 softmax/LN):
  - LayerNorm in token-major layout; gain/beta folded into projection
    weights/biases (W' = diag(g) @ W, b' = beta @ W + b) so the normalized
    activations can be PE-transposed once and used directly.
  - q/k projections produce transposed outputs [D_out, tokens]; v is
    token-major with a ones column appended per head so the A@V matmul also
    yields the softmax denominator for free.
  - Scores are computed transposed [Sk, Sq]; exp (no max subtraction --
    scores are ~N(0,1) after scaling, bounded well inside f32 range) writes
    bf16 "E^T" tiles. A@V uses v as the stationary operand and E^T moving
    (N=512 matmuls), accumulating out^T [65, 1024] per head in PSUM; row 64
    is the softmax denominator, applied via reciprocal + partition-broadcast
    + multiply, writing attn_out^T directly in the out-proj layout.
  - All work pools are shared across the k/v/q paths so the paths pipeline
    into each other instead of serializing on SBUF address reuse.
"""

import os
import numpy as np

D = 1024
H = 16
HD = 64
S = 2048
B = 4
SQ = 1024  # query rows per core
SK = 2048  # kv rows per core
KT = D // 128  # contraction tiles
MT = D // 128  # output chunks
N_CORES = 8
EPS = 1e-5

_CACHE = {}


def _build():
    from contextlib import ExitStack

    import concourse.bacc as bacc
    import concourse.bass as bass
    import concourse.mybir as mybir
    import concourse.tile as tile
    from concourse.masks import make_identity

    f32 = mybir.dt.float32
    bf16 = mybir.dt.bfloat16
    AF = mybir.ActivationFunctionType
    OP = mybir.AluOpType

    nc = bacc.Bacc("TRN2", target_bir_lowering=False, debug=False,
                   num_devices=N_CORES)

    xq = nc.dram_tensor("xq", [SQ, D], f32, kind="ExternalInput").ap()
    xk = nc.dram_tensor("xk", [SK, D], f32, kind="ExternalInput").ap()
    xv = nc.dram_tensor("xv", [SK, D], f32, kind="ExternalInput").ap()
    Wq_d = nc.dram_tensor("Wq", [D, D], f32, kind="ExternalInput").ap()
    Wk_d = nc.dram_tensor("Wk", [D, D], f32, kind="ExternalInput").ap()
    Wv_d = nc.dram_tensor("Wv", [D, D], f32, kind="ExternalInput").ap()
    Wo_d = nc.dram_tensor("Wo", [D, D], f32, kind="ExternalInput").ap()
    Wg_d = nc.dram_tensor("Wg", [D, 1], f32, kind="ExternalInput").ap()
    bq_d = nc.dram_tensor("bq", [1, D], f32, kind="ExternalInput").ap()
    bk_d = nc.dram_tensor("bk", [1, D], f32, kind="ExternalInput").ap()
    bv_d = nc.dram_tensor("bv", [1, D], f32, kind="ExternalInput").ap()
    bo_d = nc.dram_tensor("bo", [1, D], f32, kind="ExternalInput").ap()
    lnqg_d = nc.dram_tensor("lnqg", [KT, 128], f32, kind="ExternalInput").ap()
    lnqb_d = nc.dram_tensor("lnqb", [KT, 128], f32, kind="ExternalInput").ap()
    lnkg_d = nc.dram_tensor("lnkg", [KT, 128], f32, kind="ExternalInput").ap()
    lnkb_d = nc.dram_tensor("lnkb", [KT, 128], f32, kind="ExternalInput").ap()
    lnog_d = nc.dram_tensor("lnog", [1, D], f32, kind="ExternalInput").ap()
    lnob_d = nc.dram_tensor("lnob", [1, D], f32, kind="ExternalInput").ap()
    out_d = nc.dram_tensor("out", [SQ, D], f32, kind="ExternalOutput").ap()

    def bcast_rows(ap2d, p):
        return bass.AP(tensor=ap2d.tensor, offset=ap2d.offset,
                       ap=[[0, p]] + list(ap2d.ap[1:]))

    with tile.TileContext(nc) as tc:
        with ExitStack() as ctx:
            const = ctx.enter_context(tc.tile_pool(name="const", bufs=1))
            main = ctx.enter_context(tc.tile_pool(name="main", bufs=1))
            wop = ctx.enter_context(tc.tile_pool(name="wo", bufs=1))

            identity = const.tile([128, 128], bf16)
            make_identity(nc, identity)
            ones_row = const.tile([1, 512], bf16)
            nc.vector.memset(ones_row, 1.0)
            eps_t = const.tile([128, 1], f32)
            nc.vector.memset(eps_t, EPS)

            lnqg = const.tile([128, KT], f32)
            nc.sync.dma_start(out=lnqg, in_=lnqg_d.rearrange("k p -> p k"))
            lnqb = const.tile([128, KT], f32)
            nc.sync.dma_start(out=lnqb, in_=lnqb_d.rearrange("k p -> p k"))
            lnkg = const.tile([128, KT], f32)
            nc.sync.dma_start(out=lnkg, in_=lnkg_d.rearrange("k p -> p k"))
            lnkb = const.tile([128, KT], f32)
            nc.sync.dma_start(out=lnkb, in_=lnkb_d.rearrange("k p -> p k"))
            lnqb_h = const.tile([128, KT], bf16)
            nc.vector.tensor_copy(out=lnqb_h, in_=lnqb)
            lnkb_h = const.tile([128, KT], bf16)
            nc.vector.tensor_copy(out=lnkb_h, in_=lnkb)

            # persistent per-core intermediates
            kT_s = main.tile([128, MT, SK], bf16)
            qT_s = main.tile([128, MT, SQ], bf16)
            v_aug = main.tile([128, SK // 128, H, HD + 1], bf16)
            gate_s = main.tile([128, SQ // 128], f32)
            nc.vector.memset(v_aug[:, :, :, HD:HD + 1], 1.0)
            # zero-padded qT staging (one per head parity): streaming K=128
            # keeps the PE activity monitor at full clock (K=64 matmuls get
            # permanently throttled to half rate).
            qtz = [main.tile([128, SQ], bf16, name=f"qtz{i}")
                   for i in range(2)]
            nc.vector.memset(qtz[0], 0.0)
            nc.vector.memset(qtz[1], 0.0)

            Wo_s = wop.tile([128, KT, D], bf16)
            bo_s = wop.tile([1, D], bf16)

            # ---------------- pre-attention ----------------
            with tc.tile_pool(name="wst", bufs=2) as wst, \
                    tc.tile_pool(name="wpl", bufs=2) as wpl, \
                    tc.tile_pool(name="xnt", bufs=3) as xnt, \
                    tc.tile_pool(name="lnw", bufs=3) as lnw, \
                    tc.tile_pool(name="bps", bufs=1, space="PSUM") as bps, \
                    tc.tile_pool(name="lps", bufs=2, space="PSUM") as lps, \
                    tc.tile_pool(name="pps", bufs=2, space="PSUM") as pps:

                def load_weight_folded(w_dram, b_dram, g, beta_h, name,
                                       Ws=None, bs=None):
                    """W' = diag(g) W (bf16), b' = beta @ W + b."""
                    if Ws is None:
                        Ws = wpl.tile([128, KT, D], bf16, tag="W",
                                      name=f"{name}_W")
                        bs = wpl.tile([1, D], bf16, tag="b", name=f"{name}_b")
                    bp = bps.tile([1, 2, 512], f32, tag="bp",
                                  name=f"{name}_bp")
                    for kt in range(KT):
                        wc = wst.tile([128, D], f32, tag="wc",
                                      name=f"{name}_wc{kt}")
                        nc.sync.dma_start(
                            out=wc, in_=w_dram[kt * 128:(kt + 1) * 128, :])
                        if g is not None:
                            raw = wst.tile([128, D], bf16, tag="raw",
                                           bufs=1, name=f"{name}_raw{kt}")
                            nc.scalar.copy(out=raw, in_=wc)
                            nc.scalar.activation(out=Ws[:, kt, :], in_=wc,
                                                 func=AF.Copy,
                                                 scale=g[:, kt:kt + 1])
                            for n in range(2):
                                nc.tensor.matmul(
                                    out=bp[:, n, :],
                                    lhsT=beta_h[:, kt:kt + 1],
                                    rhs=raw[:, n * 512:(n + 1) * 512],
                                    start=(kt == 0), stop=(kt == KT - 1))
                        else:
                            nc.scalar.activation(out=Ws[:, kt, :], in_=wc,
                                                 func=AF.Copy)
                    bb = wst.tile([1, D], f32, tag="bb", name=f"{name}_bb")
                    nc.sync.dma_start(out=bb, in_=b_dram)
                    if g is not None:
                        nc.vector.tensor_add(
                            out=bs, in0=bp.rearrange("p a b -> p (a b)"),
                            in1=bb)
                    else:
                        nc.vector.tensor_copy(out=bs, in_=bb)
                    return Ws, bs

                def ln_transpose(x_dram, n_tok, name):
                    """LN (no gain/beta) + PE transpose into per-512-token
                    chunk tiles [128, KT, 512] bf16."""
                    chunks = [xnt.tile([128, KT, 512], bf16, tag="xnt",
                                       name=f"{name}{i}")
                              for i in range(n_tok // 512)]
                    for t in range(n_tok // 128):
                        xt = lnw.tile([128, D], f32, tag="x")
                        nc.sync.dma_start(
                            out=xt, in_=x_dram[t * 128:(t + 1) * 128, :])
                        xt3 = xt.rearrange("p (s f) -> p s f", s=2)
                        stats = lnw.tile([128, 2, 6], f32, tag="st")
                        nc.vector.bn_stats(out=stats[:, 0, :], in_=xt3[:, 0, :])
                        nc.vector.bn_stats(out=stats[:, 1, :], in_=xt3[:, 1, :])
                        mv = lnw.tile([128, 2], f32, tag="mv")
                        nc.vector.bn_aggr(out=mv, in_=stats)
                        rstd = lnw.tile([128, 1], f32, tag="rs")
                        nc.scalar.activation(out=rstd, in_=mv[:, 1:2],
                                             func=AF.Sqrt, bias=eps_t)
                        nc.vector.reciprocal(out=rstd, in_=rstd)
                        xc = lnw.tile([128, D], bf16, tag="xc", bufs=2)
                        nc.vector.tensor_scalar(
                            out=xc, in0=xt, scalar1=mv[:, 0:1], scalar2=rstd,
                            op0=OP.subtract, op1=OP.mult)
                        pt = lps.tile([128, KT, 128], bf16)
                        for c in range(KT):
                            nc.tensor.transpose(
                                out=pt[:, c, :],
                                in_=xc[:, c * 128:(c + 1) * 128],
                                identity=identity)
                        cc, col = t // 4, (t % 4) * 128
                        nc.scalar.copy(
                            out=chunks[cc][:, :, col:col + 128], in_=pt)
                    return chunks

                def proj_T(xT_chunks, Ws, bs, n_tok, dst):
                    """dst[:, m, n-chunk] = (W' xn^T + b'), bf16, transposed."""
                    for n in range(n_tok // 512):
                        xT = xT_chunks[n]
                        for m in range(MT):
                            ps = pps.tile([128, 512], f32, tag="pj")
                            for kt in range(KT):
                                nc.tensor.matmul(
                                    out=ps,
                                    lhsT=Ws[:, kt, m * 128:(m + 1) * 128],
                                    rhs=xT[:, kt, :],
                                    start=(kt == 0), stop=False)
                            nc.tensor.matmul(
                                out=ps, lhsT=bs[:, m * 128:(m + 1) * 128],
                                rhs=ones_row, start=False, stop=True)
                            nc.scalar.copy(
                                out=dst[:, m, n * 512:(n + 1) * 512], in_=ps)

                # K path
                Wk_s, bk_s = load_weight_folded(Wk_d, bk_d, lnkg, lnkb_h, "wk")
                knT = ln_transpose(xk, SK, "knT")
                proj_T(knT, Wk_s, bk_s, SK, kT_s)

                # V path
                Wv_s, bv_s = load_weight_folded(Wv_d, bv_d, lnkg, lnkb_h, "wv")
                vnT = ln_transpose(xv, SK, "vnT")
                for tt in range(SK // 128):
                    for n in range(2):
                        ps = pps.tile([128, 512], f32, tag="pj")
                        for kt in range(KT):
                            nc.tensor.matmul(
                                out=ps,
                                lhsT=vnT[tt // 4][
                                    :, kt, (tt % 4) * 128:(tt % 4 + 1) * 128],
                                rhs=Wv_s[:, kt, n * 512:(n + 1) * 512],
                                start=(kt == 0), stop=False)
                        nc.tensor.matmul(
                            out=ps, lhsT=ones_row[:, 0:128],
                            rhs=bv_s[:, n * 512:(n + 1) * 512],
                            start=False, stop=True)
                        nc.vector.tensor_copy(
                            out=v_aug[:, tt, n * 8:(n + 1) * 8, 0:HD],
                            in_=ps.rearrange("p (h d) -> p h d", h=8))

                # Q path (+ gate)
                Wq_s, bq_s = load_weight_folded(Wq_d, bq_d, lnqg, lnqb_h, "wq")
                Wg_s = const.tile([128, KT, 1], bf16)
                bg_s = const.tile([1, 1], bf16)
                bgp = bps.tile([1, 1], f32, tag="bg")
                for kt in range(KT):
                    gc = wst.tile([128, 1], f32, tag="gc", name=f"gc{kt}")
                    nc.sync.dma_start(
                        out=gc, in_=Wg_d[kt * 128:(kt + 1) * 128, :])
                    gr = wst.tile([128, 1], bf16, tag="gr", name=f"gr{kt}")
                    nc.vector.tensor_copy(out=gr, in_=gc)
                    nc.scalar.activation(out=Wg_s[:, kt, :], in_=gc,
                                         func=AF.Copy,
                                         scale=lnqg[:, kt:kt + 1])
                    nc.tensor.matmul(out=bgp, lhsT=lnqb_h[:, kt:kt + 1],
                                     rhs=gr, start=(kt == 0),
                                     stop=(kt == KT - 1))
                nc.vector.tensor_copy(out=bg_s, in_=bgp)

                qnT = ln_transpose(xq, SQ, "qnT")
                proj_T(qnT, Wq_s, bq_s, SQ, qT_s)
                for tt in range(SQ // 128):
                    ps = pps.tile([128, 1], f32, tag="g", bufs=1)
                    for kt in range(KT):
                        nc.tensor.matmul(
                            out=ps,
                            lhsT=qnT[tt // 4][
                                :, kt, (tt % 4) * 128:(tt % 4 + 1) * 128],
                            rhs=Wg_s[:, kt, :],
                            start=(kt == 0), stop=False)
                    nc.tensor.matmul(
                        out=ps, lhsT=ones_row[:, 0:128], rhs=bg_s,
                        start=False, stop=True)
                    nc.scalar.activation(
                        out=gate_s[:, tt:tt + 1], in_=ps, func=AF.Sigmoid)

                # output projection weights (no LN folding)
                load_weight_folded(Wo_d, bo_d, None, None, "wo",
                                   Ws=Wo_s, bs=bo_s)

            # ---------------- attention ----------------
            aop = ctx.enter_context(tc.tile_pool(name="aop", bufs=1))
            attn_oT = aop.tile([128, KT, SQ], bf16)
            with tc.tile_pool(name="psS", bufs=1, space="PSUM") as psS, \
                    tc.tile_pool(name="psO", bufs=2, space="PSUM") as psO, \
                    tc.tile_pool(name="et", bufs=3) as etp, \
                    tc.tile_pool(name="dv", bufs=2) as dvp:
                for h in range(H):
                    mch = h // 2
                    poh = (h % 2) * HD
                    ET = [etp.tile([128, 8, SQ], bf16, tag="et",
                                   name=f"et_h{h}_{i}") for i in range(2)]
                    pO = psO.tile([65, 2, 512], f32, tag="o",
                                  name=f"po_h{h}")
                    qz = qtz[h % 2]
                    nc.vector.tensor_copy(out=qz[poh:poh + HD, :],
                                          in_=qT_s[poh:poh + HD, mch, :])
                    for half in range(2):
                        for pl in range(4):
                            ps = psS.tile([128, 2, 8, 128], f32, tag="s",
                                          name=f"ps_h{h}_{half}_{pl}")
                            for sub in range(2):
                                sk = half * 8 + pl * 2 + sub
                                for n in range(SQ // 512):
                                    nc.tensor.matmul(
                                        out=ps[:, sub, n * 4:(n + 1) * 4, :],
                                        lhsT=kT_s[:, mch,
                                                  sk * 128:(sk + 1) * 128],
                                        rhs=qz[:, n * 512:(n + 1) * 512],
                                        start=True, stop=True)
                            nc.scalar.activation(
                                out=ET[half][:, pl * 2:pl * 2 + 2, :]
                                .rearrange("p a (b c) -> p a b c", b=8),
                                in_=ps, func=AF.Exp, scale=0.125)
                        for si in range(8):
                            sk = half * 8 + si
                            for n in range(2):
                                nc.tensor.matmul(
                                    out=pO[:, n, :],
                                    lhsT=v_aug[:, sk, h, :],
                                    rhs=ET[half][:, si,
                                                 n * 512:(n + 1) * 512],
                                    start=(sk == 0), stop=(sk == 15))
                    # softmax denominator: row 64 of pO
                    rs = dvp.tile([1, SQ], f32, tag="rs", name=f"rs_h{h}")
                    nc.vector.reciprocal(
                        out=rs, in_=pO[64:65, :, :].rearrange(
                            "p a b -> p (a b)"))
                    rb = dvp.tile([HD, SQ], f32, tag="rb", name=f"rb_h{h}")
                    nc.gpsimd.partition_broadcast(out_ap=rb, in_ap=rs)
                    nc.vector.tensor_mul(
                        out=attn_oT[poh:poh + HD, mch, :],
                        in0=pO[0:64, :, :].rearrange("p a b -> p (a b)"),
                        in1=rb)

            # ---------------- out-proj + gate + final LN ----------------
            with tc.tile_pool(name="o_ps", bufs=4, space="PSUM") as pps, \
                    tc.tile_pool(name="o_w", bufs=3) as work:
                lnog_b = work.tile([128, D], f32, tag="lng", bufs=1)
                nc.sync.dma_start(out=lnog_b, in_=bcast_rows(lnog_d, 128))
                lnob_b = work.tile([128, D], f32, tag="lnb", bufs=1)
                nc.sync.dma_start(out=lnob_b, in_=bcast_rows(lnob_d, 128))
                for tt in range(SQ // 128):
                    og = work.tile([128, D], f32, tag="og")
                    for n in range(2):
                        ps = pps.tile([128, 512], f32)
                        for kt in range(KT):
                            nc.tensor.matmul(
                                out=ps,
                                lhsT=attn_oT[:, kt, tt * 128:(tt + 1) * 128],
                                rhs=Wo_s[:, kt, n * 512:(n + 1) * 512],
                                start=(kt == 0), stop=False)
                        nc.tensor.matmul(
                            out=ps, lhsT=ones_row[:, 0:128],
                            rhs=bo_s[:, n * 512:(n + 1) * 512],
                            start=False, stop=True)
                        nc.vector.tensor_scalar_mul(
                            out=og[:, n * 512:(n + 1) * 512], in0=ps,
                            scalar1=gate_s[:, tt:tt + 1])
                    og3 = og.rearrange("p (s f) -> p s f", s=2)
                    stats = work.tile([128, 2, 6], f32, tag="st2")
                    nc.vector.bn_stats(out=stats[:, 0, :], in_=og3[:, 0, :])
                    nc.vector.bn_stats(out=stats[:, 1, :], in_=og3[:, 1, :])
                    mv = work.tile([128, 2], f32, tag="mv2")
                    nc.vector.bn_aggr(out=mv, in_=stats)
                    rstd = work.tile([128, 1], f32, tag="rs2")
                    nc.scalar.activation(out=rstd, in_=mv[:, 1:2],
                                         func=AF.Sqrt, bias=eps_t)
                    nc.vector.reciprocal(out=rstd, in_=rstd)
                    xc = work.tile([128, D], f32, tag="xc2")
                    nc.vector.tensor_scalar(
                        out=xc, in0=og, scalar1=mv[:, 0:1], scalar2=rstd,
                        op0=OP.subtract, op1=OP.mult)
                    res = work.tile([128, D], f32, tag="res")
                    nc.vector.tensor_mul(out=res, in0=xc, in1=lnog_b)
                    nc.vector.tensor_add(out=res, in0=res, in1=lnob_b)
                    nc.sync.dma_start(
                        out=out_d[tt * 128:(tt + 1) * 128, :], in_=res)

    nc.compile()
    return nc


def _maybe_enable_trace():
    """Install the axon NTFF profile hook if tracing was requested."""
    if not os.environ.get("BASS_KERNEL_TRACE"):
        return False
    try:
        import sys
        import types
        import antenv
        if "antenv.axon_hooks" not in sys.modules:
            mod = types.ModuleType("antenv.axon_hooks")
            mod._hook = None
            mod.set_axon_ntff_profile_hook = lambda h: setattr(mod, "_hook", h)
            mod.get_axon_ntff_profile_hook = lambda: mod._hook
            sys.modules["antenv.axon_hooks"] = mod
            antenv.axon_hooks = mod
        from antenv.axon_hooks import get_axon_ntff_profile_hook
        if get_axon_ntff_profile_hook() is None:
            from trn_agent_boot.trn_boot import _ntff_profile_via_ctypes
            from antenv.axon_hooks import set_axon_ntff_profile_hook
            set_axon_ntff_profile_hook(
                _ntff_profile_via_ctypes("/opt/axon/libaxon_pjrt.so"))
        return True
    except Exception:
        return False


def kernel(**inputs):
    from concourse import bass_utils

    if "nc" not in _CACHE:
        _CACHE["nc"] = _build()
    nc = _CACHE["nc"]

    f = lambda k: np.ascontiguousarray(np.asarray(inputs[k], dtype=np.float32))
    query, key, value = f("query"), f("key"), f("value")
    shared = {
        "Wq": f("Wq"), "Wk": f("Wk"), "Wv": f("Wv"), "Wo": f("Wo"),
        "Wg": f("Wg").reshape(D, 1),
        "bq": f("bq").reshape(1, D), "bk": f("bk").reshape(1, D),
        "bv": f("bv").reshape(1, D), "bo": f("bo").reshape(1, D),
        "lnqg": f("ln_q_g").reshape(KT, 128),
        "lnqb": f("ln_q_b").reshape(KT, 128),
        "lnkg": f("ln_kv_g").reshape(KT, 128),
        "lnkb": f("ln_kv_b").reshape(KT, 128),
        "lnog": f("ln_o_g").reshape(1, D),
        "lnob": f("ln_o_b").reshape(1, D),
    }
    in_maps = []
    for c in range(N_CORES):
        b, hh = c // 2, c % 2
        in_maps.append({
            "xq": np.ascontiguousarray(query[b, hh * SQ:(hh + 1) * SQ, :]),
            "xk": np.ascontiguousarray(key[b]),
            "xv": np.ascontiguousarray(value[b]),
            **shared,
        })

    trace = _maybe_enable_trace()
    kw = {}
    if trace:
        kw = dict(trace=True, trace_cores=[0])
    res = bass_utils.run_bass_kernel_spmd(
        nc, in_maps, core_ids=list(range(N_CORES)), **kw)
    if trace:
        _CACHE["exec_time_ns"] = res.exec_time_ns
        _CACHE["trace_path"] = (res.instructions_and_trace[1]
                                if res.instructions_and_trace else None)

    out = np.empty((B, S, D), dtype=np.float32)
    for c in range(N_CORES):
        b, hh = c // 2, c % 2
        out[b, hh * SQ:(hh + 1) * SQ, :] = res.results[c]["out"]
    return out


# revision 20
# speedup vs baseline: 1.2807x; 1.2807x over previous
"""EnhancedMultiHeadAttention on 8 TRN2 NeuronCores.

Sharding: core c handles batch b=c//2 and query-row half h=c%2.
Each core computes the full attention for its 1024 query rows against its
batch's full 2048 keys/values (k/v work duplicated across the 2 cores that
share a batch — cheaper than an all-reduce). Outputs are disjoint slices of
the full [4, 2048, 1024] result, assembled on the host.

Kernel structure per core (bf16 matmuls, f32 softmax/LN):
  - LayerNorm in token-major layout; gain/beta folded into projection
    weights/biases (W' = diag(g) @ W, b' = beta @ W + b) so the normalized
    activations can be PE-transposed once and used directly.
  - q/k projections produce transposed outputs [D_out, tokens]; v is
    token-major with a ones column appended per head so the A@V matmul also
    yields the softmax denominator for free.
  - Scores are computed transposed [Sk, Sq]; exp (no max subtraction --
    scores are ~N(0,1) after scaling, bounded well inside f32 range) writes
    bf16 "E^T" tiles. A@V uses v as the stationary operand and E^T moving
    (N=512 matmuls), accumulating out^T [65, 1024] per head in PSUM; row 64
    is the softmax denominator, applied via reciprocal + partition-broadcast
    + multiply, writing attn_out^T directly in the out-proj layout.
  - All work pools are shared across the k/v/q paths so the paths pipeline
    into each other instead of serializing on SBUF address reuse.
"""

import os
import numpy as np

D = 1024
H = 16
HD = 64
S = 2048
B = 4
SQ = 1024  # query rows per core
SK = 2048  # kv rows per core
KT = D // 128  # contraction tiles
MT = D // 128  # output chunks
N_CORES = 8
EPS = 1e-5

_CACHE = {}


def _build():
    from contextlib import ExitStack

    import concourse.bacc as bacc
    import concourse.bass as bass
    import concourse.mybir as mybir
    import concourse.tile as tile
    from concourse.masks import make_identity

    f32 = mybir.dt.float32
    bf16 = mybir.dt.bfloat16
    AF = mybir.ActivationFunctionType
    OP = mybir.AluOpType

    nc = bacc.Bacc("TRN2", target_bir_lowering=False, debug=False,
                   num_devices=N_CORES)

    xq = nc.dram_tensor("xq", [SQ, D], f32, kind="ExternalInput").ap()
    xk = nc.dram_tensor("xk", [SK, D], f32, kind="ExternalInput").ap()
    xv = nc.dram_tensor("xv", [SK, D], f32, kind="ExternalInput").ap()
    Wq_d = nc.dram_tensor("Wq", [D, D], f32, kind="ExternalInput").ap()
    Wk_d = nc.dram_tensor("Wk", [D, D], f32, kind="ExternalInput").ap()
    Wv_d = nc.dram_tensor("Wv", [D, D], f32, kind="ExternalInput").ap()
    Wo_d = nc.dram_tensor("Wo", [D, D], f32, kind="ExternalInput").ap()
    Wg_d = nc.dram_tensor("Wg", [D, 1], f32, kind="ExternalInput").ap()
    bq_d = nc.dram_tensor("bq", [1, D], f32, kind="ExternalInput").ap()
    bk_d = nc.dram_tensor("bk", [1, D], f32, kind="ExternalInput").ap()
    bv_d = nc.dram_tensor("bv", [1, D], f32, kind="ExternalInput").ap()
    bo_d = nc.dram_tensor("bo", [1, D], f32, kind="ExternalInput").ap()
    lnqg_d = nc.dram_tensor("lnqg", [KT, 128], f32, kind="ExternalInput").ap()
    lnqb_d = nc.dram_tensor("lnqb", [KT, 128], f32, kind="ExternalInput").ap()
    lnkg_d = nc.dram_tensor("lnkg", [KT, 128], f32, kind="ExternalInput").ap()
    lnkb_d = nc.dram_tensor("lnkb", [KT, 128], f32, kind="ExternalInput").ap()
    lnog_d = nc.dram_tensor("lnog", [1, D], f32, kind="ExternalInput").ap()
    lnob_d = nc.dram_tensor("lnob", [1, D], f32, kind="ExternalInput").ap()
    out_d = nc.dram_tensor("out", [SQ, D], f32, kind="ExternalOutput").ap()

    def bcast_rows(ap2d, p):
        return bass.AP(tensor=ap2d.tensor, offset=ap2d.offset,
                       ap=[[0, p]] + list(ap2d.ap[1:]))

    with tile.TileContext(nc) as tc:
        with ExitStack() as ctx:
            const = ctx.enter_context(tc.tile_pool(name="const", bufs=1))
            main = ctx.enter_context(tc.tile_pool(name="main", bufs=1))
            wop = ctx.enter_context(tc.tile_pool(name="wo", bufs=1))

            identity = const.tile([128, 128], bf16)
            make_identity(nc, identity)
            ones_row = const.tile([1, 512], bf16)
            nc.vector.memset(ones_row, 1.0)
            eps_t = const.tile([128, 1], f32)
            nc.vector.memset(eps_t, EPS)

            lnqg = const.tile([128, KT], f32)
            nc.sync.dma_start(out=lnqg, in_=lnqg_d.rearrange("k p -> p k"))
            lnqb = const.tile([128, KT], f32)
            nc.sync.dma_start(out=lnqb, in_=lnqb_d.rearrange("k p -> p k"))
            lnkg = const.tile([128, KT], f32)
            nc.sync.dma_start(out=lnkg, in_=lnkg_d.rearrange("k p -> p k"))
            lnkb = const.tile([128, KT], f32)
            nc.sync.dma_start(out=lnkb, in_=lnkb_d.rearrange("k p -> p k"))
            lnqb_h = const.tile([128, KT], bf16)
            nc.vector.tensor_copy(out=lnqb_h, in_=lnqb)
            lnkb_h = const.tile([128, KT], bf16)
            nc.vector.tensor_copy(out=lnkb_h, in_=lnkb)

            # persistent per-core intermediates
            kT_s = main.tile([128, MT, SK], bf16)
            qT_s = main.tile([128, MT, SQ], bf16)
            v_aug = main.tile([128, SK // 128, H, HD + 1], bf16)
            gate_s = main.tile([128, SQ // 128], f32)
            nc.vector.memset(v_aug[:, :, :, HD:HD + 1], 1.0)
            # zero-padded qT staging (one per head parity): streaming K=128
            # keeps the PE activity monitor at full clock (K=64 matmuls get
            # permanently throttled to half rate).
            qtz = [main.tile([128, SQ], bf16, name=f"qtz{i}")
                   for i in range(2)]
            nc.vector.memset(qtz[0], 0.0)
            nc.vector.memset(qtz[1], 0.0)

            Wo_s = wop.tile([128, KT, D], bf16)
            bo_s = wop.tile([1, D], bf16)

            # ---------------- pre-attention ----------------
            with tc.tile_pool(name="wst", bufs=2) as wst, \
                    tc.tile_pool(name="wpl", bufs=2) as wpl, \
                    tc.tile_pool(name="xnt", bufs=3) as xnt, \
                    tc.tile_pool(name="lnw", bufs=3) as lnw, \
                    tc.tile_pool(name="bps", bufs=1, space="PSUM") as bps, \
                    tc.tile_pool(name="lps", bufs=2, space="PSUM") as lps, \
                    tc.tile_pool(name="pps", bufs=2, space="PSUM") as pps:

                def load_weight_folded(w_dram, b_dram, g, beta_h, name,
                                       Ws=None, bs=None):
                    """W' = diag(g) W (bf16), b' = beta @ W + b."""
                    if Ws is None:
                        Ws = wpl.tile([128, KT, D], bf16, tag="W",
                                      name=f"{name}_W")
                        bs = wpl.tile([1, D], bf16, tag="b", name=f"{name}_b")
                    bp = bps.tile([1, 2, 512], f32, tag="bp",
                                  name=f"{name}_bp")
                    for kt in range(KT):
                        wc = wst.tile([128, D], f32, tag="wc",
                                      name=f"{name}_wc{kt}")
                        nc.sync.dma_start(
                            out=wc, in_=w_dram[kt * 128:(kt + 1) * 128, :])
                        if g is not None:
                            raw = wst.tile([128, D], bf16, tag="raw",
                                           bufs=1, name=f"{name}_raw{kt}")
                            nc.scalar.copy(out=raw, in_=wc)
                            nc.scalar.activation(out=Ws[:, kt, :], in_=wc,
                                                 func=AF.Copy,
                                                 scale=g[:, kt:kt + 1])
                            for n in range(2):
                                nc.tensor.matmul(
                                    out=bp[:, n, :],
                                    lhsT=beta_h[:, kt:kt + 1],
                                    rhs=raw[:, n * 512:(n + 1) * 512],
                                    start=(kt == 0), stop=(kt == KT - 1))
                        else:
                            nc.scalar.activation(out=Ws[:, kt, :], in_=wc,
                                                 func=AF.Copy)
                    bb = wst.tile([1, D], f32, tag="bb", name=f"{name}_bb")
                    nc.sync.dma_start(out=bb, in_=b_dram)
                    if g is not None:
                        nc.vector.tensor_add(
                            out=bs, in0=bp.rearrange("p a b -> p (a b)"),
                            in1=bb)
                    else:
                        nc.vector.tensor_copy(out=bs, in_=bb)
                    return Ws, bs

                def ln_transpose(x_dram, n_tok, name):
                    """LN (no gain/beta) + PE transpose into per-512-token
                    chunk tiles [128, KT, 512] bf16."""
                    chunks = [xnt.tile([128, KT, 512], bf16, tag="xnt",
                                       name=f"{name}{i}")
                              for i in range(n_tok // 512)]
                    for t in range(n_tok // 128):
                        xt = lnw.tile([128, D], f32, tag="x")
                        nc.sync.dma_start(
                            out=xt, in_=x_dram[t * 128:(t + 1) * 128, :])
                        xt3 = xt.rearrange("p (s f) -> p s f", s=2)
                        stats = lnw.tile([128, 2, 6], f32, tag="st")
                        nc.vector.bn_stats(out=stats[:, 0, :], in_=xt3[:, 0, :])
                        nc.vector.bn_stats(out=stats[:, 1, :], in_=xt3[:, 1, :])
                        mv = lnw.tile([128, 2], f32, tag="mv")
                        nc.vector.bn_aggr(out=mv, in_=stats)
                        rstd = lnw.tile([128, 1], f32, tag="rs")
                        nc.scalar.activation(out=rstd, in_=mv[:, 1:2],
                                             func=AF.Sqrt, bias=eps_t)
                        nc.vector.reciprocal(out=rstd, in_=rstd)
                        xc = lnw.tile([128, D], bf16, tag="xc", bufs=2)
                        nc.vector.tensor_scalar(
                            out=xc, in0=xt, scalar1=mv[:, 0:1], scalar2=rstd,
                            op0=OP.subtract, op1=OP.mult)
                        pt = lps.tile([128, KT, 128], bf16)
                        for c in range(KT):
                            nc.tensor.transpose(
                                out=pt[:, c, :],
                                in_=xc[:, c * 128:(c + 1) * 128],
                                identity=identity)
                        cc, col = t // 4, (t % 4) * 128
                        nc.scalar.copy(
                            out=chunks[cc][:, :, col:col + 128], in_=pt)
                    return chunks

                def proj_T(xT_chunks, Ws, bs, n_tok, dst):
                    """dst[:, m, n-chunk] = (W' xn^T + b'), bf16, transposed."""
                    for n in range(n_tok // 512):
                        xT = xT_chunks[n]
                        for m in range(MT):
                            ps = pps.tile([128, 512], f32, tag="pj")
                            for kt in range(KT):
                                nc.tensor.matmul(
                                    out=ps,
                                    lhsT=Ws[:, kt, m * 128:(m + 1) * 128],
                                    rhs=xT[:, kt, :],
                                    start=(kt == 0), stop=False)
                            nc.tensor.matmul(
                                out=ps, lhsT=bs[:, m * 128:(m + 1) * 128],
                                rhs=ones_row, start=False, stop=True)
                            nc.scalar.copy(
                                out=dst[:, m, n * 512:(n + 1) * 512], in_=ps)

                # K path
                Wk_s, bk_s = load_weight_folded(Wk_d, bk_d, lnkg, lnkb_h, "wk")
                knT = ln_transpose(xk, SK, "knT")
                proj_T(knT, Wk_s, bk_s, SK, kT_s)

                # V path
                Wv_s, bv_s = load_weight_folded(Wv_d, bv_d, lnkg, lnkb_h, "wv")
                vnT = ln_transpose(xv, SK, "vnT")
                for tt in range(SK // 128):
                    for n in range(2):
                        ps = pps.tile([128, 512], f32, tag="pj")
                        for kt in range(KT):
                            nc.tensor.matmul(
                                out=ps,
                                lhsT=vnT[tt // 4][
                                    :, kt, (tt % 4) * 128:(tt % 4 + 1) * 128],
                                rhs=Wv_s[:, kt, n * 512:(n + 1) * 512],
                                start=(kt == 0), stop=False)
                        nc.tensor.matmul(
                            out=ps, lhsT=ones_row[:, 0:128],
                            rhs=bv_s[:, n * 512:(n + 1) * 512],
                            start=False, stop=True)
                        nc.vector.tensor_copy(
                            out=v_aug[:, tt, n * 8:(n + 1) * 8, 0:HD],
                            in_=ps.rearrange("p (h d) -> p h d", h=8))

                # Q path (+ gate)
                Wq_s, bq_s = load_weight_folded(Wq_d, bq_d, lnqg, lnqb_h, "wq")
                Wg_s = const.tile([128, KT, 1], bf16)
                bg_s = const.tile([1, 1], bf16)
                bgp = bps.tile([1, 1], f32, tag="bg")
                for kt in range(KT):
                    gc = wst.tile([128, 1], f32, tag="gc", name=f"gc{kt}")
                    nc.sync.dma_start(
                        out=gc, in_=Wg_d[kt * 128:(kt + 1) * 128, :])
                    gr = wst.tile([128, 1], bf16, tag="gr", name=f"gr{kt}")
                    nc.vector.tensor_copy(out=gr, in_=gc)
                    nc.scalar.activation(out=Wg_s[:, kt, :], in_=gc,
                                         func=AF.Copy,
                                         scale=lnqg[:, kt:kt + 1])
                    nc.tensor.matmul(out=bgp, lhsT=lnqb_h[:, kt:kt + 1],
                                     rhs=gr, start=(kt == 0),
                                     stop=(kt == KT - 1))
                nc.vector.tensor_copy(out=bg_s, in_=bgp)

                qnT = ln_transpose(xq, SQ, "qnT")
                proj_T(qnT, Wq_s, bq_s, SQ, qT_s)
                for tt in range(SQ // 128):
                    ps = pps.tile([128, 1], f32, tag="g", bufs=1)
                    for kt in range(KT):
                        nc.tensor.matmul(
                            out=ps,
                            lhsT=qnT[tt // 4][
                                :, kt, (tt % 4) * 128:(tt % 4 + 1) * 128],
                            rhs=Wg_s[:, kt, :],
                            start=(kt == 0), stop=False)
                    nc.tensor.matmul(
                        out=ps, lhsT=ones_row[:, 0:128], rhs=bg_s,
                        start=False, stop=True)
                    nc.scalar.activation(
                        out=gate_s[:, tt:tt + 1], in_=ps, func=AF.Sigmoid)

                # output projection weights (no LN folding)
                load_weight_folded(Wo_d, bo_d, None, None, "wo",
                                   Ws=Wo_s, bs=bo_s)

            # ---------------- attention ----------------
            aop = ctx.enter_context(tc.tile_pool(name="aop", bufs=1))
            attn_oT = aop.tile([128, KT, SQ], bf16)
            with tc.tile_pool(name="psS", bufs=2, space="PSUM") as psS, \
                    tc.tile_pool(name="psO", bufs=2, space="PSUM") as psO, \
                    tc.tile_pool(name="et", bufs=3) as etp, \
                    tc.tile_pool(name="dv", bufs=2) as dvp:
                for h in range(H):
                    mch = h // 2
                    poh = (h % 2) * HD
                    ET = [etp.tile([128, 8, SQ], bf16, tag="et",
                                   name=f"et_h{h}_{i}") for i in range(2)]
                    pO = psO.tile([65, 2, 512], f32, tag="o",
                                  name=f"po_h{h}")
                    qz = qtz[h % 2]
                    nc.vector.tensor_copy(out=qz[poh:poh + HD, :],
                                          in_=qT_s[poh:poh + HD, mch, :])
                    for half in range(2):
                        for si in range(8):
                            sk = half * 8 + si
                            ps = psS.tile([128, SQ], f32, tag="s",
                                          name=f"ps_h{h}_{sk}")
                            for n in range(SQ // 512):
                                nc.tensor.matmul(
                                    out=ps[:, n * 512:(n + 1) * 512],
                                    lhsT=kT_s[:, mch,
                                              sk * 128:(sk + 1) * 128],
                                    rhs=qz[:, n * 512:(n + 1) * 512],
                                    start=True, stop=True)
                            nc.scalar.activation(
                                out=ET[half][:, si, :], in_=ps, func=AF.Exp,
                                scale=0.125)
                        for si in range(8):
                            sk = half * 8 + si
                            for n in range(2):
                                nc.tensor.matmul(
                                    out=pO[:, n, :],
                                    lhsT=v_aug[:, sk, h, :],
                                    rhs=ET[half][:, si,
                                                 n * 512:(n + 1) * 512],
                                    start=(sk == 0), stop=(sk == 15))
                    # softmax denominator: row 64 of pO
                    rs = dvp.tile([1, SQ], f32, tag="rs", name=f"rs_h{h}")
                    nc.vector.reciprocal(
                        out=rs, in_=pO[64:65, :, :].rearrange(
                            "p a b -> p (a b)"))
                    rb = dvp.tile([HD, SQ], f32, tag="rb", name=f"rb_h{h}")
                    nc.gpsimd.partition_broadcast(out_ap=rb, in_ap=rs)
                    nc.vector.tensor_mul(
                        out=attn_oT[poh:poh + HD, mch, :],
                        in0=pO[0:64, :, :].rearrange("p a b -> p (a b)"),
                        in1=rb)

            # ---------------- out-proj + gate + final LN ----------------
            with tc.tile_pool(name="o_ps", bufs=4, space="PSUM") as pps, \
                    tc.tile_pool(name="o_w", bufs=3) as work:
                lnog_b = work.tile([128, D], f32, tag="lng", bufs=1)
                nc.sync.dma_start(out=lnog_b, in_=bcast_rows(lnog_d, 128))
                lnob_b = work.tile([128, D], f32, tag="lnb", bufs=1)
                nc.sync.dma_start(out=lnob_b, in_=bcast_rows(lnob_d, 128))
                for tt in range(SQ // 128):
                    og = work.tile([128, D], f32, tag="og")
                    for n in range(2):
                        ps = pps.tile([128, 512], f32)
                        for kt in range(KT):
                            nc.tensor.matmul(
                                out=ps,
                                lhsT=attn_oT[:, kt, tt * 128:(tt + 1) * 128],
                                rhs=Wo_s[:, kt, n * 512:(n + 1) * 512],
                                start=(kt == 0), stop=False)
                        nc.tensor.matmul(
                            out=ps, lhsT=ones_row[:, 0:128],
                            rhs=bo_s[:, n * 512:(n + 1) * 512],
                            start=False, stop=True)
                        nc.vector.tensor_scalar_mul(
                            out=og[:, n * 512:(n + 1) * 512], in0=ps,
                            scalar1=gate_s[:, tt:tt + 1])
                    og3 = og.rearrange("p (s f) -> p s f", s=2)
                    stats = work.tile([128, 2, 6], f32, tag="st2")
                    nc.vector.bn_stats(out=stats[:, 0, :], in_=og3[:, 0, :])
                    nc.vector.bn_stats(out=stats[:, 1, :], in_=og3[:, 1, :])
                    mv = work.tile([128, 2], f32, tag="mv2")
                    nc.vector.bn_aggr(out=mv, in_=stats)
                    rstd = work.tile([128, 1], f32, tag="rs2")
                    nc.scalar.activation(out=rstd, in_=mv[:, 1:2],
                                         func=AF.Sqrt, bias=eps_t)
                    nc.vector.reciprocal(out=rstd, in_=rstd)
                    xc = work.tile([128, D], f32, tag="xc2")
                    nc.vector.tensor_scalar(
                        out=xc, in0=og, scalar1=mv[:, 0:1], scalar2=rstd,
                        op0=OP.subtract, op1=OP.mult)
                    res = work.tile([128, D], f32, tag="res")
                    nc.vector.tensor_mul(out=res, in0=xc, in1=lnog_b)
                    nc.vector.tensor_add(out=res, in0=res, in1=lnob_b)
                    nc.sync.dma_start(
                        out=out_d[tt * 128:(tt + 1) * 128, :], in_=res)

    nc.compile()
    return nc


def _maybe_enable_trace():
    """Install the axon NTFF profile hook if tracing was requested."""
    if not os.environ.get("BASS_KERNEL_TRACE"):
        return False
    try:
        import sys
        import types
        import antenv
        if "antenv.axon_hooks" not in sys.modules:
            mod = types.ModuleType("antenv.axon_hooks")
            mod._hook = None
            mod.set_axon_ntff_profile_hook = lambda h: setattr(mod, "_hook", h)
            mod.get_axon_ntff_profile_hook = lambda: mod._hook
            sys.modules["antenv.axon_hooks"] = mod
            antenv.axon_hooks = mod
        from antenv.axon_hooks import get_axon_ntff_profile_hook
        if get_axon_ntff_profile_hook() is None:
            from trn_agent_boot.trn_boot import _ntff_profile_via_ctypes
            from antenv.axon_hooks import set_axon_ntff_profile_hook
            set_axon_ntff_profile_hook(
                _ntff_profile_via_ctypes("/opt/axon/libaxon_pjrt.so"))
        return True
    except Exception:
        return False


def kernel(**inputs):
    from concourse import bass_utils

    if "nc" not in _CACHE:
        _CACHE["nc"] = _build()
    nc = _CACHE["nc"]

    f = lambda k: np.ascontiguousarray(np.asarray(inputs[k], dtype=np.float32))
    query, key, value = f("query"), f("key"), f("value")
    shared = {
        "Wq": f("Wq"), "Wk": f("Wk"), "Wv": f("Wv"), "Wo": f("Wo"),
        "Wg": f("Wg").reshape(D, 1),
        "bq": f("bq").reshape(1, D), "bk": f("bk").reshape(1, D),
        "bv": f("bv").reshape(1, D), "bo": f("bo").reshape(1, D),
        "lnqg": f("ln_q_g").reshape(KT, 128),
        "lnqb": f("ln_q_b").reshape(KT, 128),
        "lnkg": f("ln_kv_g").reshape(KT, 128),
        "lnkb": f("ln_kv_b").reshape(KT, 128),
        "lnog": f("ln_o_g").reshape(1, D),
        "lnob": f("ln_o_b").reshape(1, D),
    }
    in_maps = []
    for c in range(N_CORES):
        b, hh = c // 2, c % 2
        in_maps.append({
            "xq": np.ascontiguousarray(query[b, hh * SQ:(hh + 1) * SQ, :]),
            "xk": np.ascontiguousarray(key[b]),
            "xv": np.ascontiguousarray(value[b]),
            **shared,
        })

    trace = _maybe_enable_trace()
    kw = {}
    if trace:
        kw = dict(trace=True, trace_cores=[0])
    res = bass_utils.run_bass_kernel_spmd(
        nc, in_maps, core_ids=list(range(N_CORES)), **kw)
    if trace:
        _CACHE["exec_time_ns"] = res.exec_time_ns
        _CACHE["trace_path"] = (res.instructions_and_trace[1]
                                if res.instructions_and_trace else None)

    out = np.empty((B, S, D), dtype=np.float32)
    for c in range(N_CORES):
        b, hh = c // 2, c % 2
        out[b, hh * SQ:(hh + 1) * SQ, :] = res.results[c]["out"]
    return out
